# revision 23
# baseline (speedup 1.0000x reference)
"""BiLSTM-CRF Trainium2 kernel (8-core SPMD, batch-sharded).

Per core: 4 sequences, full pipeline on device:
  embedding gather (indirect DMA) -> PE transposes -> input-gate GEMMs ->
  512-step bidirectional LSTM recurrence -> emission GEMM ->
  chunked Viterbi max-plus scan (32 chunks x 16 steps, parallel across
  128 partitions) -> pointer-map suffix composition -> chunk-boundary
  walks -> int32 tag path.

Math notes:
  sigmoid(x) = 0.5*tanh(0.5x)+0.5 so every gate uses one Tanh activation; the
  0.5 factors are pre-folded into the weights. Cell/hidden state are carried
  doubled (C=2c, H=2h); the 0.5 for H is folded into W_hh and W_out.

  Viterbi runs in restricted tag spaces: next-tag j in {0..8} (START/STOP/PAD
  rows can never win, margins ~1e4), prev-tag k in {0..8, START} where the
  START slot is NEG everywhere except the t=0 entry vector. Pointers and tag
  maps are carried as w-values w = 11 - tag so a reduce_max implements
  first-index argmax exactly like jnp.argmax.
"""

import numpy as np

import concourse.bass as bass
import concourse.tile as tile
from concourse import bacc, mybir
from concourse.bass_utils import run_bass_kernel_spmd

FP = mybir.dt.float32
FH = mybir.dt.float16
I32 = mybir.dt.int32
AX = mybir.AxisListType
OP = mybir.AluOpType
AF = mybir.ActivationFunctionType

VOCAB = 100000
E = 256
Hh = 128
K = 12
START = 9
STOP = 10
NEG = -10000.0
B = 32
NCORES = 8
BL = B // NCORES  # 4 sequences per core
JN = 9            # next-tag slots: tags 0..8
KN = 10           # prev-tag slots: tags 0..8 + START
CL = 16           # viterbi chunk length


def build_program(T=512):
    nc = bacc.Bacc("TRN2", target_bir_lowering=False, debug=False)
    NTOK = T * BL              # tokens per core
    NTILE = NTOK // 128        # gather tiles (16 at T=512)
    NCHUNK = NTOK // 512       # 512-col GEMM chunks (4)
    NC = T // CL               # viterbi chunks (32)

    def din(name, shape, dtype=FP):
        return nc.dram_tensor(name, list(shape), dtype, kind="ExternalInput").ap()

    idx_in = din("idx_in", [128, NTILE], I32)          # [p,k] token ids, time-major
    embed = din("embed", [VOCAB, E])
    w_ihT = din("w_ihT", [2, E, 4 * Hh], FH)           # pre-scaled, gate order i,f,o,g
    w_hhT = din("w_hhT", [2, Hh, 4 * Hh], FH)
    b_in = din("b_in", [128, 8])                       # col d*4+g: per-partition bias
    h_init = din("h_init", [128, 2 * BL], FH)          # 2*h0, feature-major, (d,b)
    c_init = din("c_init", [128, 2 * BL])              # 2*c0, (d,b)
    w_outT = din("w_outT", [2, Hh, JN], FH)            # 0.5*W_out halves (tags 0..8)
    bout_rep = din("bout_rep", [128, JN])
    ident = din("ident", [128, 128])
    identh = din("identh", [128, 128], FH)
    a10 = din("a10", [128, JN * KN])                   # trans[j, k10] replicated
    wvk = din("wvk", [128, KN])                        # 11-k for k-slots
    wvj = din("wvj", [128, JN])                        # 11-j for j-slots
    tstop9 = din("tstop9", [BL, JN])                   # trans[STOP, 0:9] replicated

    path_out = nc.dram_tensor("path_out", [BL, T], I32, kind="ExternalOutput").ap()

    # DRAM scratch for partition-permute bounces
    f128_d = nc.dram_tensor("f128_d", [NTILE, 128, JN], FP).ap()
    ft2_d = nc.dram_tensor("ft2_d", [BL, T // CL, CL, JN], FP).ap()
    p_d = nc.dram_tensor("p_d", [128, KN * JN], FP).ap()
    s_d = nc.dram_tensor("s_d", [BL, NC, KN], FP).ap()
    fc_d = nc.dram_tensor("fc_d", [128, JN], FP).ap()
    te_d = nc.dram_tensor("te_d", [BL, NC], FP).ap()

    with tile.TileContext(nc) as tc:
        with tc.tile_pool(name="const", bufs=1) as cpool, \
             tc.tile_pool(name="big", bufs=1) as bpool:

            # ---- load constants ----
            def cload(ap_in, shape, dtype=FP):
                t = cpool.tile(list(shape), dtype, name=f"c_{np.random.randint(1 << 30)}")
                nc.sync.dma_start(t[:], ap_in)
                return t

            idx_sb = cload(idx_in, [128, NTILE], I32)
            wih_sb = [[cload(w_ihT[d, e * 128:(e + 1) * 128, :], [128, 4 * Hh], FH)
                       for e in range(2)] for d in range(2)]
            whh_sb = [cload(w_hhT[d], [Hh, 4 * Hh], FH) for d in range(2)]
            b_sb = cload(b_in, [128, 8])
            hi_sb = cload(h_init, [128, 2 * BL], FH)
            ci_sb = cload(c_init, [128, 2 * BL])
            wout_sb = [cload(w_outT[d], [Hh, JN], FH) for d in range(2)]
            bout_sb = cload(bout_rep, [128, JN])
            id_sb = cload(ident, [128, 128])
            idh_sb = cload(identh, [128, 128], FH)
            a10_sb = cload(a10, [128, JN * KN])
            wvk_sb = cload(wvk, [128, KN])
            wvj_sb = cload(wvj, [128, JN])
            ts_sb = cload(tstop9, [BL, JN])

            # big persistent arrays (fp16: matmul operands)
            xg_sb = [bpool.tile([128, T * 16], FH, tag=f"xg{d}", name=f"xg{d}") for d in range(2)]
            hs_sb = [bpool.tile([128, T * BL], FH, tag=f"hs{d}", name=f"hs{d}") for d in range(2)]

            # ---- phase 1: embedding gather + transpose to [E, tok] ----
            with tc.tile_pool(name="gat", bufs=3) as gpool, \
                 tc.tile_pool(name="ps1", bufs=4, space="PSUM") as ps1, \
                 tc.tile_pool(name="xe", bufs=1) as xepool:
                xe_sb = [xepool.tile([128, NTOK], FH, tag=f"xe{e}", name=f"xe{e}") for e in range(2)]
                for k in range(NTILE):
                    gt = gpool.tile([128, E], FP)
                    nc.gpsimd.indirect_dma_start(
                        out=gt[:],
                        out_offset=None,
                        in_=embed[:],
                        in_offset=bass.IndirectOffsetOnAxis(
                            ap=idx_sb[:, k:k + 1], axis=0),
                    )
                    for e in range(2):
                        pt = ps1.tile([128, 128], FP, space="PSUM")
                        nc.tensor.transpose(
                            out=pt[:], in_=gt[:, e * 128:(e + 1) * 128],
                            identity=id_sb[:])
                        nc.vector.tensor_copy(
                            xe_sb[e][:, k * 128:(k + 1) * 128], pt[:])

                # ---- phase 2: xg = W_ih_eff @ xe + b, interleaved [t,(g,b)] ----
                with tc.tile_pool(name="ps2", bufs=3, space="PSUM") as ps2:
                    for d in range(2):
                        xgv = xg_sb[d][:].rearrange("p (t x) -> p t x", x=16)
                        for g in range(4):
                            for c in range(NCHUNK):
                                pt = ps2.tile([128, 512], FP, space="PSUM")
                                for e in range(2):
                                    nc.tensor.matmul(
                                        pt[:],
                                        lhsT=wih_sb[d][e][:, g * 128:(g + 1) * 128],
                                        rhs=xe_sb[e][:, c * 512:(c + 1) * 512],
                                        start=(e == 0), stop=(e == 1),
                                    )
                                nc.vector.tensor_scalar(
                                    out=xgv[:, c * 128:(c + 1) * 128,
                                            g * 4:(g + 1) * 4],
                                    in0=pt[:].rearrange("p (t b) -> p t b", b=BL),
                                    scalar1=b_sb[:, d * 4 + g:d * 4 + g + 1],
                                    scalar2=None,
                                    op0=OP.add,
                                )

            # ---- phase 3: LSTM recurrence, both directions fused per op ----
            # psum/th cols (d, g, b): per dir i=0:4, f=4:8, o=8:12, g=12:16
            with tc.tile_pool(name="ps3", bufs=4, space="PSUM") as ps3, \
                 tc.tile_pool(name="th", bufs=4) as thpool, \
                 tc.tile_pool(name="cell", bufs=4) as cellpool, \
                 tc.tile_pool(name="cst", bufs=2) as cstpool:
                c_cur = ci_sb
                for step in range(T):
                    tt = [step, T - 1 - step]
                    prev = [hi_sb[:, d * BL:(d + 1) * BL] if step == 0 else
                            hs_sb[d][:, (tt[d] - 1 + 2 * d) * BL:
                                      (tt[d] + 2 * d) * BL]
                            for d in range(2)]
                    p = ps3.tile([128, 32], FP, space="PSUM",
                                 tag="g", name=f"g_{step}")
                    for d in range(2):
                        for q in range(4):
                            nc.tensor.matmul(
                                p[32 * q:32 * (q + 1), d * 16:(d + 1) * 16],
                                lhsT=idh_sb[:, 32 * q:32 * (q + 1)],
                                rhs=xg_sb[d][:, tt[d] * 16:(tt[d] + 1) * 16],
                                start=True, stop=False,
                                tile_position=(0, 32 * q),
                                skip_group_check=True)
                    for d in range(2):
                        for g in range(4):
                            for q in range(4):
                                nc.tensor.matmul(
                                    p[32 * q:32 * (q + 1),
                                      d * 16 + g * 4:d * 16 + (g + 1) * 4],
                                    lhsT=whh_sb[d][:, g * 128 + 32 * q:
                                                   g * 128 + 32 * (q + 1)],
                                    rhs=prev[d],
                                    start=False, stop=(g == 3 and q == 3),
                                    tile_position=(0, 32 * q),
                                    skip_group_check=True)
                    th = thpool.tile([128, 32], FP, tag="th",
                                     name=f"th_{step}")
                    nc.scalar.activation(th[:], p[:], AF.Tanh)
                    th4 = th[:].rearrange("p (d g b) -> p d g b", d=2, g=4)
                    a_t = cellpool.tile([128, 2 * BL], FP, tag="a",
                                        name=f"a_{step}")
                    b_t = cellpool.tile([128, 2 * BL], FP, tag="b",
                                        name=f"b_{step}")
                    nc.vector.scalar_tensor_tensor(
                        out=a_t[:].rearrange("p (d o b) -> p d o b", d=2, o=1),
                        in0=th4[:, :, 1:2, :], scalar=1.0,
                        in1=c_cur[:].rearrange("p (d o b) -> p d o b",
                                               d=2, o=1),
                        op0=OP.add, op1=OP.mult)
                    nc.vector.scalar_tensor_tensor(
                        out=b_t[:].rearrange("p (d o b) -> p d o b", d=2, o=1),
                        in0=th4[:, :, 0:1, :], scalar=1.0,
                        in1=th4[:, :, 3:4, :], op0=OP.add, op1=OP.mult)
                    c_n = cstpool.tile([128, 2 * BL], FP, tag="c",
                                       name=f"c_{step}")
                    nc.vector.scalar_tensor_tensor(
                        out=c_n[:], in0=a_t[:], scalar=0.5,
                        in1=b_t[:], op0=OP.mult, op1=OP.add)
                    tc_t = cellpool.tile([128, 2 * BL], FP, tag="tc",
                                         name=f"tc_{step}")
                    nc.scalar.activation(tc_t[:], c_n[:], AF.Tanh, scale=0.5)
                    for d in range(2):
                        nc.vector.scalar_tensor_tensor(
                            out=hs_sb[d][:, tt[d] * BL:(tt[d] + 1) * BL],
                            in0=th[:, d * 16 + 8:d * 16 + 12], scalar=1.0,
                            in1=tc_t[:, d * BL:(d + 1) * BL],
                            op0=OP.add, op1=OP.mult)
                    c_cur = c_n

            # ---- phase 4: emission scores (tags 0..8) -> f128_d ----
            with tc.tile_pool(name="ps4", bufs=3, space="PSUM") as ps4, \
                 tc.tile_pool(name="fsb", bufs=3) as fpool:
                for ch in range(NTILE):
                    pt = ps4.tile([128, JN], FP, space="PSUM")
                    for d in range(2):
                        nc.tensor.matmul(
                            pt[:],
                            lhsT=hs_sb[d][:, ch * 128:(ch + 1) * 128],
                            rhs=wout_sb[d][:],
                            start=(d == 0), stop=(d == 1))
                    fsb = fpool.tile([128, JN], FP)
                    nc.vector.tensor_add(fsb[:], pt[:], bout_sb[:])
                    nc.sync.dma_start(f128_d[ch], fsb[:])

            # ---- phase 5: chunked Viterbi ----
            # partition p = b*NC + cc  (4 seqs x 32 chunks)
            with tc.tile_pool(name="vit", bufs=1) as vp:
                FT = vp.tile([128, CL * JN], FP, name="FT")        # feats [p,(l,j)]
                nc.sync.dma_start(
                    ft2_d.rearrange("b (c th) l j -> c th l b j", th=2),
                    f128_d.rearrange("c (th l b) j -> c th l b j",
                                     th=2, l=CL, b=BL))
                nc.sync.dma_start(
                    FT[:], ft2_d.rearrange("b cc l j -> (b cc) (l j)"))
                a3 = a10_sb[:].rearrange("p (j k) -> p j k", k=KN)  # [p,j,k10]

                # pass 1: chunk matrices P[j,i] stored [p,(j,i)]
                ptA = vp.tile([128, JN * KN], FP, name="ptA")
                ptB = vp.tile([128, JN * KN], FP, name="ptB")
                tmp = vp.tile([128, JN * KN * JN], FP, name="tmp")  # [p,(j,i,k9)]
                # init: P0[j,i] = A[j,i] + f0[j]
                nc.vector.tensor_tensor(
                    out=ptA[:].rearrange("p (j i) -> p j i", i=KN),
                    in0=a3,
                    in1=FT[:, 0:JN].unsqueeze(2).broadcast_to([128, JN, KN]),
                    op=OP.add)
                cur, nxt = ptA, ptB
                for l in range(1, CL):
                    nc.vector.tensor_tensor(
                        out=tmp[:].rearrange("p (j i k) -> p j i k", i=KN, k=JN),
                        in0=a3[:, :, 0:JN].unsqueeze(2)
                            .broadcast_to([128, JN, KN, JN]),
                        in1=cur[:].rearrange("p (k i) -> p k i", i=KN)
                            .transpose([0, 2, 1]).unsqueeze(1)
                            .broadcast_to([128, JN, KN, JN]),
                        op=OP.add)
                    nc.vector.reduce_max(
                        nxt[:].rearrange("p (j i) -> p j i", i=KN),
                        tmp[:].rearrange("p (j i k) -> p j i k", i=KN, k=JN),
                        axis=AX.X)
                    nc.vector.tensor_tensor(
                        out=nxt[:].rearrange("p (j i) -> p j i", i=KN),
                        in0=nxt[:].rearrange("p (j i) -> p j i", i=KN),
                        in1=FT[:, l * JN:(l + 1) * JN].unsqueeze(2)
                            .broadcast_to([128, JN, KN]),
                        op=OP.add)
                    cur, nxt = nxt, cur
                nc.sync.dma_start(p_d, cur[:])

                # pass 2: boundary walk on [BL, *]
                PW = vp.tile([BL, NC * JN * KN], FP, name="PW")
                nc.sync.dma_start(
                    PW[:], p_d.rearrange("(b cc) x -> b (cc x)", b=BL))
                SENT = vp.tile([BL, (NC + 1) * KN], FP, name="SENT")
                nc.vector.memset(SENT[:], NEG)
                nc.vector.memset(SENT[:, START:START + 1], 0.0)
                pwv = PW[:].rearrange("b (cc j i) -> b cc j i", j=JN, i=KN)
                sev = SENT[:].rearrange("b (cc k) -> b cc k", k=KN)
                tw = vp.tile([BL, JN * KN], FP, name="tw")
                for c in range(NC):
                    nc.vector.tensor_tensor(
                        out=tw[:].rearrange("b (o j i) -> b o j i", o=1, i=KN),
                        in0=pwv[:, c:c + 1],
                        in1=sev[:, c:c + 1].unsqueeze(2)
                            .broadcast_to([BL, 1, JN, KN]),
                        op=OP.add)
                    nc.vector.reduce_max(
                        SENT[:, (c + 1) * KN:(c + 1) * KN + JN],
                        tw[:].rearrange("b (o j i) -> b o j i", o=1, i=KN),
                        axis=AX.X)
                # best tag (w-value)
                fin = vp.tile([BL, JN], FP, name="fin")
                nc.vector.tensor_add(fin[:], SENT[:, NC * KN:NC * KN + JN],
                                     ts_sb[:])
                mx8 = vp.tile([BL, 8], FP, name="mx8")
                nc.vector.max(mx8[:], fin[:])
                nc.vector.tensor_scalar(
                    out=fin[:], in0=fin[:], scalar1=mx8[:, 0:1], scalar2=None,
                    op0=OP.is_equal)
                nc.vector.tensor_mul(fin[:], fin[:], wvj_sb[0:BL, :])
                bw = vp.tile([BL, 1], FP, name="bw")
                nc.vector.reduce_max(bw[:], fin[:].rearrange("b (o j) -> b o j", o=1),
                                     axis=AX.X)
                nc.sync.dma_start(
                    s_d, SENT[:, 0:NC * KN].rearrange("b (cc k) -> b cc k",
                                                      k=KN))

                # pass 3: within-chunk scan + pointer extraction (w-values)
                SC = vp.tile([128, KN], FP, name="SC")
                nc.sync.dma_start(
                    SC[:], s_d.rearrange("b cc k -> (b cc) k"))
                PTR = vp.tile([128, CL * JN], FP, name="PTR")
                m3 = vp.tile([128, JN * KN], FP, name="m3")
                mx = vp.tile([128, JN], FP, name="mxs")
                m3v = m3[:].rearrange("p (j k) -> p j k", k=KN)
                for l in range(CL):
                    nc.vector.tensor_tensor(
                        out=m3v,
                        in0=SC[:].unsqueeze(1).broadcast_to([128, JN, KN]),
                        in1=a3, op=OP.add)
                    nc.vector.reduce_max(mx[:], m3v, axis=AX.X)
                    nc.vector.tensor_tensor(
                        out=m3v, in0=m3v,
                        in1=mx[:].unsqueeze(2).broadcast_to([128, JN, KN]),
                        op=OP.is_equal)
                    nc.vector.tensor_tensor(
                        out=m3v, in0=m3v,
                        in1=wvk_sb[:].unsqueeze(1).broadcast_to([128, JN, KN]),
                        op=OP.mult)
                    nc.vector.reduce_max(PTR[:, l * JN:(l + 1) * JN], m3v,
                                         axis=AX.X)
                    nc.vector.tensor_add(SC[:, 0:JN], mx[:],
                                         FT[:, l * JN:(l + 1) * JN])

                # bt pass 1: suffix compositions SUF[l] and chunk map FC
                SUF = vp.tile([128, CL * JN], FP, name="SUF")
                FC = vp.tile([128, JN], FP, name="FC")
                oh = vp.tile([128, JN * JN], FP, name="oh")
                ohv = oh[:].rearrange("p (j i) -> p j i", i=JN)
                nc.vector.tensor_copy(SUF[:, (CL - 1) * JN:CL * JN],
                                      wvj_sb[:])
                for l in range(CL - 2, -2, -1):
                    src = SUF[:, (l + 1) * JN:(l + 2) * JN]
                    dst = FC[:] if l == -1 else SUF[:, l * JN:(l + 1) * JN]
                    nc.vector.tensor_tensor(
                        out=ohv,
                        in0=src.unsqueeze(2).broadcast_to([128, JN, JN]),
                        in1=wvj_sb[:].unsqueeze(1).broadcast_to([128, JN, JN]),
                        op=OP.is_equal)
                    nc.vector.tensor_tensor(
                        out=ohv, in0=ohv,
                        in1=PTR[:, (l + 1) * JN:(l + 2) * JN]
                            .unsqueeze(1).broadcast_to([128, JN, JN]),
                        op=OP.mult)
                    nc.vector.reduce_max(dst, ohv, axis=AX.X)
                nc.sync.dma_start(fc_d, FC[:])

                # tag walk on [BL, *]
                FW = vp.tile([BL, NC * JN], FP, name="FW")
                nc.sync.dma_start(
                    FW[:], fc_d.rearrange("(b cc) j -> b (cc j)", b=BL))
                TE = vp.tile([BL, NC], FP, name="TE")
                ohw = vp.tile([BL, JN], FP, name="ohw")
                scr = vp.tile([BL, JN], FP, name="scr")
                nc.vector.tensor_copy(TE[:, NC - 1:NC], bw[:])
                for c in range(NC - 1, 0, -1):
                    nc.vector.tensor_scalar(
                        out=ohw[:], in0=wvj_sb[0:BL, :],
                        scalar1=TE[:, c:c + 1], scalar2=None, op0=OP.is_equal)
                    nc.vector.scalar_tensor_tensor(
                        out=scr[:], in0=ohw[:], scalar=1.0,
                        in1=FW[:, c * JN:(c + 1) * JN],
                        op0=OP.mult, op1=OP.mult,
                        accum_out=TE[:, c - 1:c])
                nc.sync.dma_start(te_d, TE[:])

                # bt pass 3: apply suffix maps per partition
                TE128 = vp.tile([128, 1], FP, name="TE128")
                nc.sync.dma_start(
                    TE128[:], te_d.rearrange("b (cc x) -> (b cc) x", x=1))
                oh2 = vp.tile([128, JN], FP, name="oh2")
                nc.vector.tensor_scalar(
                    out=oh2[:], in0=wvj_sb[:], scalar1=TE128[:, 0:1],
                    scalar2=None, op0=OP.is_equal)
                big = vp.tile([128, CL * JN], FP, name="bigm")
                nc.vector.tensor_tensor(
                    out=big[:].rearrange("p (l j) -> p l j", j=JN),
                    in0=SUF[:].rearrange("p (l j) -> p l j", j=JN),
                    in1=oh2[:].unsqueeze(1).broadcast_to([128, CL, JN]),
                    op=OP.mult)
                pw16 = vp.tile([128, CL], FP, name="pw16")
                nc.vector.reduce_max(
                    pw16[:], big[:].rearrange("p (l j) -> p l j", j=JN),
                    axis=AX.X)
                pi = vp.tile([128, CL], I32, name="pi")
                nc.vector.tensor_scalar(
                    out=pi[:], in0=pw16[:], scalar1=-1.0, scalar2=float(K - 1),
                    op0=OP.mult, op1=OP.add)
                nc.sync.dma_start(
                    path_out.rearrange("b (cc l) -> (b cc) l", l=CL), pi[:])

    nc.compile()
    return nc


def prep_inputs(sentence, h0, c0, embed, W_ih_f, W_hh_f, b_f, W_ih_r, W_hh_r,
                b_r, W_out, b_out, transitions, T=512):
    """Host-side layout prep. Returns per-core input maps."""
    f32 = np.float32
    perm = np.r_[0:128, 128:256, 384:512, 256:384]  # i,f,g,o -> i,f,o,g
    gs = np.concatenate([np.full(128, s, f32) for s in (0.5, 0.5, 0.5, 1.0)])

    def prep_dir(W_ih, W_hh, b):
        Wi = np.asarray(W_ih, f32)[perm] * gs[:, None]
        bb = np.asarray(b, f32)[perm] * gs
        Wh = np.asarray(W_hh, f32)[perm] * (0.5 * gs)[:, None]
        return Wi.T.copy(), Wh.T.copy(), bb

    wihT_f, whhT_f, be_f = prep_dir(W_ih_f, W_hh_f, b_f)
    wihT_r, whhT_r, be_r = prep_dir(W_ih_r, W_hh_r, b_r)
    w_ihT = np.stack([wihT_f, wihT_r]).astype(np.float16)
    w_hhT = np.stack([whhT_f, whhT_r]).astype(np.float16)
    b_in = np.stack([be_f.reshape(4, 128), be_r.reshape(4, 128)])  # [2,4,128]
    b_in = b_in.reshape(8, 128).T.copy()                           # [128,8]

    Wo = np.asarray(W_out, f32)[0:JN] * 0.5                        # tags 0..8
    w_outT = np.stack([Wo[:, :128].T.copy(),
                       Wo[:, 128:].T.copy()]).astype(np.float16)
    bout_rep = np.tile(np.asarray(b_out, f32)[None, 0:JN], (128, 1))

    tr = np.asarray(transitions, f32)
    a10 = np.tile(tr[0:JN, 0:KN].reshape(1, JN * KN), (128, 1))
    wvk = np.tile((11.0 - np.arange(KN, dtype=f32))[None, :], (128, 1))
    wvj = np.tile((11.0 - np.arange(JN, dtype=f32))[None, :], (128, 1))
    tstop9 = np.tile(tr[STOP, 0:JN][None, :], (BL, 1))
    ident = np.eye(128, dtype=f32)
    embed = np.asarray(embed, f32)
    sentence = np.asarray(sentence)

    maps = []
    for core in range(NCORES):
        sl = sentence[core * BL:(core + 1) * BL, :T].astype(np.int32)
        idx_tm = sl.T.reshape(-1)                       # n = t*BL+b
        idx_in = idx_tm.reshape(-1, 128).T.copy()       # [128, NTILE]
        h_i = 2.0 * np.asarray(h0, f32)[:, core * BL:(core + 1) * BL, :]
        c_i = 2.0 * np.asarray(c0, f32)[:, core * BL:(core + 1) * BL, :]
        maps.append({
            "idx_in": idx_in,
            "embed": embed,
            "w_ihT": w_ihT,
            "w_hhT": w_hhT,
            "b_in": b_in,
            "h_init": np.ascontiguousarray(
                h_i.transpose(2, 0, 1).reshape(128, 2 * BL)).astype(np.float16),
            "c_init": np.ascontiguousarray(
                c_i.transpose(2, 0, 1).reshape(128, 2 * BL)),
            "w_outT": w_outT,
            "bout_rep": bout_rep,
            "ident": ident,
            "identh": ident.astype(np.float16),
            "a10": a10,
            "wvk": wvk,
            "wvj": wvj,
            "tstop9": tstop9,
        })
    return maps


_NC_CACHE = {}


def kernel(sentence, h0, c0, embed, W_ih_f, W_hh_f, b_f, W_ih_r, W_hh_r, b_r,
           W_out, b_out, transitions):
    T = np.asarray(sentence).shape[1]
    if T not in _NC_CACHE:
        _NC_CACHE[T] = build_program(T)
    nc = _NC_CACHE[T]
    maps = prep_inputs(sentence, h0, c0, embed, W_ih_f, W_hh_f, b_f,
                       W_ih_r, W_hh_r, b_r, W_out, b_out, transitions, T=T)
    res = run_bass_kernel_spmd(nc, maps, list(range(NCORES)))
    out = np.concatenate([res.results[i]["path_out"] for i in range(NCORES)], axis=0)
    return out.astype(np.int32)


# revision 27
# speedup vs baseline: 1.0472x; 1.0472x over previous
"""BiLSTM-CRF Trainium2 kernel (8-core SPMD, batch-sharded).

Per core: 4 sequences, full pipeline on device:
  embedding gather (indirect DMA) -> PE transposes -> input-gate GEMMs ->
  512-step bidirectional LSTM recurrence -> emission GEMM ->
  chunked Viterbi max-plus scan (32 chunks x 16 steps, parallel across
  128 partitions) -> pointer-map suffix composition -> chunk-boundary
  walks -> int32 tag path.

Math notes:
  sigmoid(x) = 0.5*tanh(0.5x)+0.5 so every gate uses one Tanh activation; the
  0.5 factors are pre-folded into the weights. Cell/hidden state are carried
  doubled (C=2c, H=2h); the 0.5 for H is folded into W_hh and W_out.

  Viterbi runs in restricted tag spaces: next-tag j in {0..8} (START/STOP/PAD
  rows can never win, margins ~1e4), prev-tag k in {0..8, START} where the
  START slot is NEG everywhere except the t=0 entry vector. Pointers and tag
  maps are carried as w-values w = 11 - tag so a reduce_max implements
  first-index argmax exactly like jnp.argmax.
"""

import numpy as np

import concourse.bass as bass
import concourse.tile as tile
from concourse import bacc, mybir
from concourse.bass_utils import run_bass_kernel_spmd

FP = mybir.dt.float32
FH = mybir.dt.float16
I32 = mybir.dt.int32
AX = mybir.AxisListType
OP = mybir.AluOpType
AF = mybir.ActivationFunctionType

VOCAB = 100000
E = 256
Hh = 128
K = 12
START = 9
STOP = 10
NEG = -10000.0
B = 32
NCORES = 8
BL = B // NCORES  # 4 sequences per core
JN = 9            # next-tag slots: tags 0..8
KN = 10           # prev-tag slots: tags 0..8 + START
CL = 16           # viterbi chunk length


def build_program(T=512):
    nc = bacc.Bacc("TRN2", target_bir_lowering=False, debug=False)
    NTOK = T * BL              # tokens per core
    NTILE = NTOK // 128        # gather tiles (16 at T=512)
    NCHUNK = NTOK // 512       # 512-col GEMM chunks (4)
    NC = T // CL               # viterbi chunks (32)

    def din(name, shape, dtype=FP):
        return nc.dram_tensor(name, list(shape), dtype, kind="ExternalInput").ap()

    idx_in = din("idx_in", [128, NTILE], I32)          # [p,k] token ids, time-major
    embed = din("embed", [VOCAB, E])
    w_ihT = din("w_ihT", [2, E, 4 * Hh], FH)           # pre-scaled, gate order i,f,o,g
    w_hhT = din("w_hhT", [2, Hh, 4 * Hh], FH)
    b_in = din("b_in", [128, 8])                       # col d*4+g: per-partition bias
    h_init = din("h_init", [2, 128, BL], FH)           # 2*h0, feature-major
    c_init = din("c_init", [2, 128, BL])               # 2*c0
    w_outT = din("w_outT", [2, Hh, JN], FH)            # 0.5*W_out halves (tags 0..8)
    bout_rep = din("bout_rep", [128, JN])
    ident = din("ident", [128, 128])
    identh = din("identh", [128, 128], FH)
    a10 = din("a10", [128, JN * KN])                   # trans[j, k10] replicated
    wvk = din("wvk", [128, KN])                        # 11-k for k-slots
    wvj = din("wvj", [128, JN])                        # 11-j for j-slots
    tstop9 = din("tstop9", [BL, JN])                   # trans[STOP, 0:9] replicated

    path_out = nc.dram_tensor("path_out", [BL, T], I32, kind="ExternalOutput").ap()

    # DRAM scratch for partition-permute bounces
    f128_d = nc.dram_tensor("f128_d", [NTILE, 128, JN], FP).ap()
    ft2_d = nc.dram_tensor("ft2_d", [BL, T // CL, CL, JN], FP).ap()
    p_d = nc.dram_tensor("p_d", [128, KN * JN], FP).ap()
    s_d = nc.dram_tensor("s_d", [BL, NC, KN], FP).ap()
    fc_d = nc.dram_tensor("fc_d", [128, JN], FP).ap()
    te_d = nc.dram_tensor("te_d", [BL, NC], FP).ap()

    with tile.TileContext(nc) as tc:
        with tc.tile_pool(name="const", bufs=1) as cpool, \
             tc.tile_pool(name="big", bufs=1) as bpool:

            # ---- load constants ----
            def cload(ap_in, shape, dtype=FP):
                t = cpool.tile(list(shape), dtype, name=f"c_{np.random.randint(1 << 30)}")
                nc.sync.dma_start(t[:], ap_in)
                return t

            idx_sb = cload(idx_in, [128, NTILE], I32)
            wih_sb = [[cload(w_ihT[d, e * 128:(e + 1) * 128, :], [128, 4 * Hh], FH)
                       for e in range(2)] for d in range(2)]
            whh_sb = [cload(w_hhT[d], [Hh, 4 * Hh], FH) for d in range(2)]
            b_sb = cload(b_in, [128, 8])
            hi_sb = [cload(h_init[d], [128, BL], FH) for d in range(2)]
            ci_sb = [cload(c_init[d], [128, BL]) for d in range(2)]
            wout_sb = [cload(w_outT[d], [Hh, JN], FH) for d in range(2)]
            bout_sb = cload(bout_rep, [128, JN])
            id_sb = cload(ident, [128, 128])
            idh_sb = cload(identh, [128, 128], FH)
            a10_sb = cload(a10, [128, JN * KN])
            wvk_sb = cload(wvk, [128, KN])
            wvj_sb = cload(wvj, [128, JN])
            ts_sb = cload(tstop9, [BL, JN])

            # big persistent arrays (fp16: matmul operands)
            xg_sb = [bpool.tile([128, T * 16], FH, tag=f"xg{d}", name=f"xg{d}") for d in range(2)]
            hs_sb = [bpool.tile([128, T * BL], FH, tag=f"hs{d}", name=f"hs{d}") for d in range(2)]

            # ---- phase 1: embedding gather + transpose to [E, tok] ----
            with tc.tile_pool(name="gat", bufs=3) as gpool, \
                 tc.tile_pool(name="ps1", bufs=4, space="PSUM") as ps1, \
                 tc.tile_pool(name="xe", bufs=1) as xepool:
                xe_sb = [xepool.tile([128, NTOK], FH, tag=f"xe{e}", name=f"xe{e}") for e in range(2)]
                for k in range(NTILE):
                    gt = gpool.tile([128, E], FP)
                    nc.gpsimd.indirect_dma_start(
                        out=gt[:],
                        out_offset=None,
                        in_=embed[:],
                        in_offset=bass.IndirectOffsetOnAxis(
                            ap=idx_sb[:, k:k + 1], axis=0),
                    )
                    for e in range(2):
                        pt = ps1.tile([128, 128], FP, space="PSUM")
                        nc.tensor.transpose(
                            out=pt[:], in_=gt[:, e * 128:(e + 1) * 128],
                            identity=id_sb[:])
                        nc.vector.tensor_copy(
                            xe_sb[e][:, k * 128:(k + 1) * 128], pt[:])

                # ---- phase 2: xg = W_ih_eff @ xe + b, interleaved [t,(g,b)] ----
                with tc.tile_pool(name="ps2", bufs=3, space="PSUM") as ps2:
                    for d in range(2):
                        xgv = xg_sb[d][:].rearrange("p (t x) -> p t x", x=16)
                        for g in range(4):
                            for c in range(NCHUNK):
                                pt = ps2.tile([128, 512], FP, space="PSUM")
                                for e in range(2):
                                    nc.tensor.matmul(
                                        pt[:],
                                        lhsT=wih_sb[d][e][:, g * 128:(g + 1) * 128],
                                        rhs=xe_sb[e][:, c * 512:(c + 1) * 512],
                                        start=(e == 0), stop=(e == 1),
                                    )
                                nc.vector.tensor_scalar(
                                    out=xgv[:, c * 128:(c + 1) * 128,
                                            g * 4:(g + 1) * 4],
                                    in0=pt[:].rearrange("p (t b) -> p t b", b=BL),
                                    scalar1=b_sb[:, d * 4 + g:d * 4 + g + 1],
                                    scalar2=None,
                                    op0=OP.add,
                                )

            # ---- phase 3: LSTM recurrence, both directions interleaved ----
            # gate cols per step: i=0:4, f=4:8, o=8:12, g=12:16
            with tc.tile_pool(name="ps3", bufs=4, space="PSUM") as ps3, \
                 tc.tile_pool(name="th", bufs=4) as thpool, \
                 tc.tile_pool(name="cell", bufs=4) as cellpool, \
                 tc.tile_pool(name="cst", bufs=2) as cstpool:
                c_cur = [ci_sb[0], ci_sb[1]]
                for step in range(T):
                    tt = [step, T - 1 - step]
                    prev = [hi_sb[d][:] if step == 0 else
                            hs_sb[d][:, (tt[d] - 1 + 2 * d) * BL:
                                      (tt[d] + 2 * d) * BL]
                            for d in range(2)]
                    # stage-major emission: engine queues alternate f/r so a
                    # stalled instruction never blocks the other chain.
                    pt = []
                    for d in range(2):
                        p = ps3.tile([128, 16], FP, space="PSUM",
                                     tag=f"g{d}", name=f"g{d}_{step}")
                        pt.append(p)
                        for q in range(4):
                            nc.tensor.matmul(
                                p[32 * q:32 * (q + 1), :],
                                lhsT=idh_sb[:, 32 * q:32 * (q + 1)],
                                rhs=xg_sb[d][:, tt[d] * 16:(tt[d] + 1) * 16],
                                start=True, stop=False,
                                tile_position=(0, 32 * q),
                                skip_group_check=True)
                    for d in range(2):
                        for g in range(4):
                            for q in range(4):
                                nc.tensor.matmul(
                                    pt[d][32 * q:32 * (q + 1), g * 4:(g + 1) * 4],
                                    lhsT=whh_sb[d][:, g * 128 + 32 * q:
                                                   g * 128 + 32 * (q + 1)],
                                    rhs=prev[d],
                                    start=False, stop=(g == 3 and q == 3),
                                    tile_position=(0, 32 * q),
                                    skip_group_check=True)
                    th = []
                    for d in range(2):
                        t_ = thpool.tile([128, 16], FP, tag=f"th{d}",
                                         name=f"th{d}_{step}")
                        th.append(t_)
                        nc.scalar.activation(t_[:], pt[d][:], AF.Tanh)
                    ab = []
                    for d in range(2):
                        a_t = cellpool.tile([128, BL], FP, tag=f"a{d}",
                                            name=f"a{d}_{step}")
                        b_t = cellpool.tile([128, BL], FP, tag=f"b{d}",
                                            name=f"b{d}_{step}")
                        nc.vector.scalar_tensor_tensor(
                            out=a_t[:], in0=th[d][:, 4:8], scalar=1.0,
                            in1=c_cur[d][:], op0=OP.add, op1=OP.mult)
                        nc.vector.scalar_tensor_tensor(
                            out=b_t[:], in0=th[d][:, 0:4], scalar=1.0,
                            in1=th[d][:, 12:16], op0=OP.add, op1=OP.mult)
                        ab.append((a_t, b_t))
                    c_new = []
                    for d in range(2):
                        c_n = cstpool.tile([128, BL], FP, tag=f"c{d}",
                                           name=f"c{d}_{step}")
                        c_new.append(c_n)
                        nc.vector.scalar_tensor_tensor(
                            out=c_n[:], in0=ab[d][0][:], scalar=0.5,
                            in1=ab[d][1][:], op0=OP.mult, op1=OP.add)
                    tc_t = []
                    for d in range(2):
                        t_ = cellpool.tile([128, BL], FP, tag=f"tc{d}",
                                           name=f"tc{d}_{step}")
                        tc_t.append(t_)
                        nc.scalar.activation(t_[:], c_new[d][:], AF.Tanh,
                                             scale=0.5)
                    for d in range(2):
                        nc.vector.scalar_tensor_tensor(
                            out=hs_sb[d][:, tt[d] * BL:(tt[d] + 1) * BL],
                            in0=th[d][:, 8:12], scalar=1.0,
                            in1=tc_t[d][:], op0=OP.add, op1=OP.mult)
                        c_cur[d] = c_new[d]

            # ---- phase 4: emission scores (tags 0..8) -> f128_d ----
            with tc.tile_pool(name="ps4", bufs=3, space="PSUM") as ps4, \
                 tc.tile_pool(name="fsb", bufs=3) as fpool:
                for ch in range(NTILE):
                    pt = ps4.tile([128, JN], FP, space="PSUM")
                    for d in range(2):
                        nc.tensor.matmul(
                            pt[:],
                            lhsT=hs_sb[d][:, ch * 128:(ch + 1) * 128],
                            rhs=wout_sb[d][:],
                            start=(d == 0), stop=(d == 1))
                    fsb = fpool.tile([128, JN], FP)
                    nc.vector.tensor_add(fsb[:], pt[:], bout_sb[:])
                    nc.sync.dma_start(f128_d[ch], fsb[:])

            # ---- phase 5: chunked Viterbi ----
            # partition p = b*NC + cc  (4 seqs x 32 chunks)
            with tc.tile_pool(name="vit", bufs=1) as vp:
                FT = vp.tile([128, CL * JN], FP, name="FT")        # feats [p,(l,j)]
                nc.sync.dma_start(
                    ft2_d.rearrange("b (c th) l j -> c th l b j", th=2),
                    f128_d.rearrange("c (th l b) j -> c th l b j",
                                     th=2, l=CL, b=BL))
                nc.sync.dma_start(
                    FT[:], ft2_d.rearrange("b cc l j -> (b cc) (l j)"))
                a3 = a10_sb[:].rearrange("p (j k) -> p j k", k=KN)  # [p,j,k10]

                # pass 1: chunk matrices P[j,i] stored [p,(j,i)]
                ptA = vp.tile([128, JN * KN], FP, name="ptA")
                ptB = vp.tile([128, JN * KN], FP, name="ptB")
                tmp = vp.tile([128, JN * KN * JN], FP, name="tmp")  # [p,(j,i,k9)]
                # init: P0[j,i] = A[j,i] + f0[j]
                nc.vector.tensor_tensor(
                    out=ptA[:].rearrange("p (j i) -> p j i", i=KN),
                    in0=a3,
                    in1=FT[:, 0:JN].unsqueeze(2).broadcast_to([128, JN, KN]),
                    op=OP.add)
                cur, nxt = ptA, ptB
                for l in range(1, CL):
                    nc.vector.tensor_tensor(
                        out=tmp[:].rearrange("p (j i k) -> p j i k", i=KN, k=JN),
                        in0=a3[:, :, 0:JN].unsqueeze(2)
                            .broadcast_to([128, JN, KN, JN]),
                        in1=cur[:].rearrange("p (k i) -> p k i", i=KN)
                            .transpose([0, 2, 1]).unsqueeze(1)
                            .broadcast_to([128, JN, KN, JN]),
                        op=OP.add)
                    nc.vector.reduce_max(
                        nxt[:].rearrange("p (j i) -> p j i", i=KN),
                        tmp[:].rearrange("p (j i k) -> p j i k", i=KN, k=JN),
                        axis=AX.X)
                    nc.vector.tensor_tensor(
                        out=nxt[:].rearrange("p (j i) -> p j i", i=KN),
                        in0=nxt[:].rearrange("p (j i) -> p j i", i=KN),
                        in1=FT[:, l * JN:(l + 1) * JN].unsqueeze(2)
                            .broadcast_to([128, JN, KN]),
                        op=OP.add)
                    cur, nxt = nxt, cur
                nc.sync.dma_start(p_d, cur[:])

                # pass 2: boundary walk on [BL, *]
                PW = vp.tile([BL, NC * JN * KN], FP, name="PW")
                nc.sync.dma_start(
                    PW[:], p_d.rearrange("(b cc) x -> b (cc x)", b=BL))
                SENT = vp.tile([BL, (NC + 1) * KN], FP, name="SENT")
                nc.vector.memset(SENT[:], NEG)
                nc.vector.memset(SENT[:, START:START + 1], 0.0)
                pwv = PW[:].rearrange("b (cc j i) -> b cc j i", j=JN, i=KN)
                sev = SENT[:].rearrange("b (cc k) -> b cc k", k=KN)
                tw = vp.tile([BL, JN * KN], FP, name="tw")
                for c in range(NC):
                    nc.vector.tensor_tensor(
                        out=tw[:].rearrange("b (o j i) -> b o j i", o=1, i=KN),
                        in0=pwv[:, c:c + 1],
                        in1=sev[:, c:c + 1].unsqueeze(2)
                            .broadcast_to([BL, 1, JN, KN]),
                        op=OP.add)
                    nc.vector.reduce_max(
                        SENT[:, (c + 1) * KN:(c + 1) * KN + JN],
                        tw[:].rearrange("b (o j i) -> b o j i", o=1, i=KN),
                        axis=AX.X)
                # best tag (w-value)
                fin = vp.tile([BL, JN], FP, name="fin")
                nc.vector.tensor_add(fin[:], SENT[:, NC * KN:NC * KN + JN],
                                     ts_sb[:])
                mx8 = vp.tile([BL, 8], FP, name="mx8")
                nc.vector.max(mx8[:], fin[:])
                nc.vector.tensor_scalar(
                    out=fin[:], in0=fin[:], scalar1=mx8[:, 0:1], scalar2=None,
                    op0=OP.is_equal)
                nc.vector.tensor_mul(fin[:], fin[:], wvj_sb[0:BL, :])
                bw = vp.tile([BL, 1], FP, name="bw")
                nc.vector.reduce_max(bw[:], fin[:].rearrange("b (o j) -> b o j", o=1),
                                     axis=AX.X)
                nc.sync.dma_start(
                    s_d, SENT[:, 0:NC * KN].rearrange("b (cc k) -> b cc k",
                                                      k=KN))

                # pass 3: within-chunk scan + pointer extraction (w-values)
                SC = vp.tile([128, KN], FP, name="SC")
                nc.sync.dma_start(
                    SC[:], s_d.rearrange("b cc k -> (b cc) k"))
                PTR = vp.tile([128, CL * JN], FP, name="PTR")
                m3 = vp.tile([128, JN * KN], FP, name="m3")
                mx = vp.tile([128, JN], FP, name="mxs")
                m3v = m3[:].rearrange("p (j k) -> p j k", k=KN)
                for l in range(CL):
                    nc.vector.tensor_tensor(
                        out=m3v,
                        in0=SC[:].unsqueeze(1).broadcast_to([128, JN, KN]),
                        in1=a3, op=OP.add)
                    nc.vector.reduce_max(mx[:], m3v, axis=AX.X)
                    nc.vector.tensor_tensor(
                        out=m3v, in0=m3v,
                        in1=mx[:].unsqueeze(2).broadcast_to([128, JN, KN]),
                        op=OP.is_equal)
                    nc.vector.tensor_tensor(
                        out=m3v, in0=m3v,
                        in1=wvk_sb[:].unsqueeze(1).broadcast_to([128, JN, KN]),
                        op=OP.mult)
                    nc.vector.reduce_max(PTR[:, l * JN:(l + 1) * JN], m3v,
                                         axis=AX.X)
                    nc.vector.tensor_add(SC[:, 0:JN], mx[:],
                                         FT[:, l * JN:(l + 1) * JN])

                # bt pass 1: suffix compositions SUF[l] and chunk map FC
                SUF = vp.tile([128, CL * JN], FP, name="SUF")
                FC = vp.tile([128, JN], FP, name="FC")
                oh = vp.tile([128, JN * JN], FP, name="oh")
                ohv = oh[:].rearrange("p (j i) -> p j i", i=JN)
                nc.vector.tensor_copy(SUF[:, (CL - 1) * JN:CL * JN],
                                      wvj_sb[:])
                for l in range(CL - 2, -2, -1):
                    src = SUF[:, (l + 1) * JN:(l + 2) * JN]
                    dst = FC[:] if l == -1 else SUF[:, l * JN:(l + 1) * JN]
                    nc.vector.tensor_tensor(
                        out=ohv,
                        in0=src.unsqueeze(2).broadcast_to([128, JN, JN]),
                        in1=wvj_sb[:].unsqueeze(1).broadcast_to([128, JN, JN]),
                        op=OP.is_equal)
                    nc.vector.tensor_tensor(
                        out=ohv, in0=ohv,
                        in1=PTR[:, (l + 1) * JN:(l + 2) * JN]
                            .unsqueeze(1).broadcast_to([128, JN, JN]),
                        op=OP.mult)
                    nc.vector.reduce_max(dst, ohv, axis=AX.X)
                nc.sync.dma_start(fc_d, FC[:])

                # tag walk on [BL, *]
                FW = vp.tile([BL, NC * JN], FP, name="FW")
                nc.sync.dma_start(
                    FW[:], fc_d.rearrange("(b cc) j -> b (cc j)", b=BL))
                TE = vp.tile([BL, NC], FP, name="TE")
                ohw = vp.tile([BL, JN], FP, name="ohw")
                scr = vp.tile([BL, JN], FP, name="scr")
                nc.vector.tensor_copy(TE[:, NC - 1:NC], bw[:])
                for c in range(NC - 1, 0, -1):
                    nc.vector.tensor_scalar(
                        out=ohw[:], in0=wvj_sb[0:BL, :],
                        scalar1=TE[:, c:c + 1], scalar2=None, op0=OP.is_equal)
                    nc.vector.scalar_tensor_tensor(
                        out=scr[:], in0=ohw[:], scalar=1.0,
                        in1=FW[:, c * JN:(c + 1) * JN],
                        op0=OP.mult, op1=OP.mult,
                        accum_out=TE[:, c - 1:c])
                nc.sync.dma_start(te_d, TE[:])

                # bt pass 3: apply suffix maps per partition
                TE128 = vp.tile([128, 1], FP, name="TE128")
                nc.sync.dma_start(
                    TE128[:], te_d.rearrange("b (cc x) -> (b cc) x", x=1))
                oh2 = vp.tile([128, JN], FP, name="oh2")
                nc.vector.tensor_scalar(
                    out=oh2[:], in0=wvj_sb[:], scalar1=TE128[:, 0:1],
                    scalar2=None, op0=OP.is_equal)
                big = vp.tile([128, CL * JN], FP, name="bigm")
                nc.vector.tensor_tensor(
                    out=big[:].rearrange("p (l j) -> p l j", j=JN),
                    in0=SUF[:].rearrange("p (l j) -> p l j", j=JN),
                    in1=oh2[:].unsqueeze(1).broadcast_to([128, CL, JN]),
                    op=OP.mult)
                pw16 = vp.tile([128, CL], FP, name="pw16")
                nc.vector.reduce_max(
                    pw16[:], big[:].rearrange("p (l j) -> p l j", j=JN),
                    axis=AX.X)
                pi = vp.tile([128, CL], I32, name="pi")
                nc.vector.tensor_scalar(
                    out=pi[:], in0=pw16[:], scalar1=-1.0, scalar2=float(K - 1),
                    op0=OP.mult, op1=OP.add)
                nc.sync.dma_start(
                    path_out.rearrange("b (cc l) -> (b cc) l", l=CL), pi[:])

    nc.compile()
    return nc


def prep_inputs(sentence, h0, c0, embed, W_ih_f, W_hh_f, b_f, W_ih_r, W_hh_r,
                b_r, W_out, b_out, transitions, T=512):
    """Host-side layout prep. Returns per-core input maps."""
    f32 = np.float32
    perm = np.r_[0:128, 128:256, 384:512, 256:384]  # i,f,g,o -> i,f,o,g
    gs = np.concatenate([np.full(128, s, f32) for s in (0.5, 0.5, 0.5, 1.0)])

    def prep_dir(W_ih, W_hh, b):
        Wi = np.asarray(W_ih, f32)[perm] * gs[:, None]
        bb = np.asarray(b, f32)[perm] * gs
        Wh = np.asarray(W_hh, f32)[perm] * (0.5 * gs)[:, None]
        return Wi.T.copy(), Wh.T.copy(), bb

    wihT_f, whhT_f, be_f = prep_dir(W_ih_f, W_hh_f, b_f)
    wihT_r, whhT_r, be_r = prep_dir(W_ih_r, W_hh_r, b_r)
    w_ihT = np.stack([wihT_f, wihT_r]).astype(np.float16)
    w_hhT = np.stack([whhT_f, whhT_r]).astype(np.float16)
    b_in = np.stack([be_f.reshape(4, 128), be_r.reshape(4, 128)])  # [2,4,128]
    b_in = b_in.reshape(8, 128).T.copy()                           # [128,8]

    Wo = np.asarray(W_out, f32)[0:JN] * 0.5                        # tags 0..8
    w_outT = np.stack([Wo[:, :128].T.copy(),
                       Wo[:, 128:].T.copy()]).astype(np.float16)
    bout_rep = np.tile(np.asarray(b_out, f32)[None, 0:JN], (128, 1))

    tr = np.asarray(transitions, f32)
    a10 = np.tile(tr[0:JN, 0:KN].reshape(1, JN * KN), (128, 1))
    wvk = np.tile((11.0 - np.arange(KN, dtype=f32))[None, :], (128, 1))
    wvj = np.tile((11.0 - np.arange(JN, dtype=f32))[None, :], (128, 1))
    tstop9 = np.tile(tr[STOP, 0:JN][None, :], (BL, 1))
    ident = np.eye(128, dtype=f32)
    embed = np.asarray(embed, f32)
    sentence = np.asarray(sentence)

    maps = []
    for core in range(NCORES):
        sl = sentence[core * BL:(core + 1) * BL, :T].astype(np.int32)
        idx_tm = sl.T.reshape(-1)                       # n = t*BL+b
        idx_in = idx_tm.reshape(-1, 128).T.copy()       # [128, NTILE]
        h_i = 2.0 * np.asarray(h0, f32)[:, core * BL:(core + 1) * BL, :]
        c_i = 2.0 * np.asarray(c0, f32)[:, core * BL:(core + 1) * BL, :]
        maps.append({
            "idx_in": idx_in,
            "embed": embed,
            "w_ihT": w_ihT,
            "w_hhT": w_hhT,
            "b_in": b_in,
            "h_init": np.ascontiguousarray(h_i.transpose(0, 2, 1)).astype(np.float16),
            "c_init": np.ascontiguousarray(c_i.transpose(0, 2, 1)),
            "w_outT": w_outT,
            "bout_rep": bout_rep,
            "ident": ident,
            "identh": ident.astype(np.float16),
            "a10": a10,
            "wvk": wvk,
            "wvj": wvj,
            "tstop9": tstop9,
        })
    return maps


_NC_CACHE = {}


def kernel(sentence, h0, c0, embed, W_ih_f, W_hh_f, b_f, W_ih_r, W_hh_r, b_r,
           W_out, b_out, transitions):
    T = np.asarray(sentence).shape[1]
    if T not in _NC_CACHE:
        _NC_CACHE[T] = build_program(T)
    nc = _NC_CACHE[T]
    maps = prep_inputs(sentence, h0, c0, embed, W_ih_f, W_hh_f, b_f,
                       W_ih_r, W_hh_r, b_r, W_out, b_out, transitions, T=T)
    res = run_bass_kernel_spmd(nc, maps, list(range(NCORES)))
    out = np.concatenate([res.results[i]["path_out"] for i in range(NCORES)], axis=0)
    return out.astype(np.int32)


# revision 35
# speedup vs baseline: 1.1980x; 1.1440x over previous
"""BiLSTM-CRF Trainium2 kernel (8-core SPMD, batch-sharded).

Per core: 4 sequences, full pipeline on device:
  embedding gather (indirect DMA) -> PE transposes -> input-gate GEMMs ->
  512-step bidirectional LSTM recurrence -> emission GEMM ->
  chunked Viterbi max-plus scan (32 chunks x 16 steps, parallel across
  128 partitions) -> pointer-map suffix composition -> chunk-boundary
  walks -> int32 tag path.

Math notes:
  sigmoid(x) = 0.5*tanh(0.5x)+0.5 so every gate uses one Tanh activation; the
  0.5 factors are pre-folded into the weights. Cell/hidden state are carried
  doubled (C=2c, H=2h); the 0.5 for H is folded into W_hh and W_out.

  Viterbi runs in restricted tag spaces: next-tag j in {0..8} (START/STOP/PAD
  rows can never win, margins ~1e4), prev-tag k in {0..8, START} where the
  START slot is NEG everywhere except the t=0 entry vector. Pointers and tag
  maps are carried as w-values w = 11 - tag so a reduce_max implements
  first-index argmax exactly like jnp.argmax.
"""

import numpy as np

import concourse.bass as bass
import concourse.tile as tile
from concourse import bacc, mybir
from concourse.bass_utils import run_bass_kernel_spmd

FP = mybir.dt.float32
FH = mybir.dt.float16
I32 = mybir.dt.int32
AX = mybir.AxisListType
OP = mybir.AluOpType
AF = mybir.ActivationFunctionType

VOCAB = 100000
E = 256
Hh = 128
K = 12
START = 9
STOP = 10
NEG = -10000.0
B = 32
NCORES = 8
BL = B // NCORES  # 4 sequences per core
JN = 9            # next-tag slots: tags 0..8
KN = 10           # prev-tag slots: tags 0..8 + START
CL = 16           # viterbi chunk length


def build_program(T=512):
    nc = bacc.Bacc("TRN2", target_bir_lowering=False, debug=False)
    NTOK = T * BL              # tokens per core
    NTILE = NTOK // 128        # gather tiles (16 at T=512)
    NCHUNK = NTOK // 512       # 512-col GEMM chunks (4)
    NC = T // CL               # viterbi chunks (32)

    def din(name, shape, dtype=FP):
        return nc.dram_tensor(name, list(shape), dtype, kind="ExternalInput").ap()

    idx_in = din("idx_in", [128, NTILE], I32)          # [p,k] token ids, time-major
    embed = din("embed", [VOCAB, E])
    w_ihT = din("w_ihT", [2, E, 4 * Hh], FH)           # pre-scaled, gate order i,f,o,g
    w_hhT = din("w_hhT", [2, Hh, 4 * Hh], FH)
    b_in = din("b_in", [128, 8])                       # col d*4+g: per-partition bias
    h_init = din("h_init", [2, 128, BL], FH)           # 2*h0, feature-major
    c_init = din("c_init", [2, 128, BL])               # 2*c0
    w_outT = din("w_outT", [2, Hh, JN], FH)            # 0.5*W_out halves (tags 0..8)
    bout_rep = din("bout_rep", [128, JN])
    ident = din("ident", [128, 128])
    identh = din("identh", [128, 128], FH)
    a10 = din("a10", [128, JN * KN])                   # trans[j, k10] replicated
    wvk = din("wvk", [128, KN])                        # 11-k for k-slots
    wvj = din("wvj", [128, JN])                        # 11-j for j-slots
    tstop9 = din("tstop9", [BL, JN])                   # trans[STOP, 0:9] replicated

    path_out = nc.dram_tensor("path_out", [BL, T], I32, kind="ExternalOutput").ap()

    # DRAM scratch for partition-permute bounces
    f128_d = nc.dram_tensor("f128_d", [NTILE, 128, JN], FP).ap()
    ft2_d = nc.dram_tensor("ft2_d", [BL, T // CL, CL, JN], FP).ap()
    p_d = nc.dram_tensor("p_d", [128, KN * JN], FP).ap()
    s_d = nc.dram_tensor("s_d", [BL, NC, KN], FP).ap()
    fc_d = nc.dram_tensor("fc_d", [128, JN], FP).ap()
    te_d = nc.dram_tensor("te_d", [BL, NC], FP).ap()

    with tile.TileContext(nc) as tc:
        with tc.tile_pool(name="const", bufs=1) as cpool, \
             tc.tile_pool(name="big", bufs=1) as bpool:

            # ---- load constants ----
            def cload(ap_in, shape, dtype=FP):
                t = cpool.tile(list(shape), dtype, name=f"c_{np.random.randint(1 << 30)}")
                nc.sync.dma_start(t[:], ap_in)
                return t

            idx_sb = cload(idx_in, [128, NTILE], I32)
            wih_sb = [[cload(w_ihT[d, e * 128:(e + 1) * 128, :], [128, 4 * Hh], FH)
                       for e in range(2)] for d in range(2)]
            whh_sb = [cload(w_hhT[d], [Hh, 4 * Hh], FH) for d in range(2)]
            b_sb = cload(b_in, [128, 8])
            hi_sb = [cload(h_init[d], [128, BL], FH) for d in range(2)]
            ci_sb = [cload(c_init[d], [128, BL]) for d in range(2)]
            wout_sb = [cload(w_outT[d], [Hh, JN], FH) for d in range(2)]
            bout_sb = cload(bout_rep, [128, JN])
            id_sb = cload(ident, [128, 128])
            idh_sb = cload(identh, [128, 128], FH)
            a10_sb = cload(a10, [128, JN * KN])
            wvk_sb = cload(wvk, [128, KN])
            wvj_sb = cload(wvj, [128, JN])
            ts_sb = cload(tstop9, [BL, JN])

            # big persistent arrays (fp16: matmul operands)
            xg_sb = [bpool.tile([128, T * 16], FH, tag=f"xg{d}", name=f"xg{d}") for d in range(2)]
            hs_sb = [bpool.tile([128, T * BL], FH, tag=f"hs{d}", name=f"hs{d}") for d in range(2)]

            # ---- phase 1: embedding gather + transpose to [E, tok] ----
            with tc.tile_pool(name="gat", bufs=3) as gpool, \
                 tc.tile_pool(name="ps1", bufs=4, space="PSUM") as ps1, \
                 tc.tile_pool(name="xe", bufs=1) as xepool:
                xe_sb = [xepool.tile([128, NTOK], FH, tag=f"xe{e}", name=f"xe{e}") for e in range(2)]
                for k in range(NTILE):
                    gt = gpool.tile([128, E], FP)
                    nc.gpsimd.indirect_dma_start(
                        out=gt[:],
                        out_offset=None,
                        in_=embed[:],
                        in_offset=bass.IndirectOffsetOnAxis(
                            ap=idx_sb[:, k:k + 1], axis=0),
                    )
                    for e in range(2):
                        pt = ps1.tile([128, 128], FP, space="PSUM")
                        nc.tensor.transpose(
                            out=pt[:], in_=gt[:, e * 128:(e + 1) * 128],
                            identity=id_sb[:])
                        nc.vector.tensor_copy(
                            xe_sb[e][:, k * 128:(k + 1) * 128], pt[:])

                # ---- phase 2: xg = W_ih_eff @ xe + b, interleaved [t,(g,b)] ----
                with tc.tile_pool(name="ps2", bufs=3, space="PSUM") as ps2:
                    for d in range(2):
                        xgv = xg_sb[d][:].rearrange("p (t x) -> p t x", x=16)
                        for g in range(4):
                            for c in range(NCHUNK):
                                pt = ps2.tile([128, 512], FP, space="PSUM")
                                for e in range(2):
                                    nc.tensor.matmul(
                                        pt[:],
                                        lhsT=wih_sb[d][e][:, g * 128:(g + 1) * 128],
                                        rhs=xe_sb[e][:, c * 512:(c + 1) * 512],
                                        start=(e == 0), stop=(e == 1),
                                    )
                                nc.vector.tensor_scalar(
                                    out=xgv[:, c * 128:(c + 1) * 128,
                                            g * 4:(g + 1) * 4],
                                    in0=pt[:].rearrange("p (t b) -> p t b", b=BL),
                                    scalar1=b_sb[:, d * 4 + g:d * 4 + g + 1],
                                    scalar2=None,
                                    op0=OP.add,
                                )

            # ---- phase 3: LSTM recurrence, both directions interleaved ----
            # gate cols per step: i=0:4, f=4:8, o=8:12, g=12:16
            with tc.tile_pool(name="ps3", bufs=4, space="PSUM") as ps3, \
                 tc.tile_pool(name="th", bufs=4) as thpool, \
                 tc.tile_pool(name="cell", bufs=4) as cellpool, \
                 tc.tile_pool(name="cst", bufs=2) as cstpool:
                c_cur = [ci_sb[0], ci_sb[1]]
                for step in range(T):
                    tt = [step, T - 1 - step]
                    prev = [hi_sb[d][:] if step == 0 else
                            hs_sb[d][:, (tt[d] - 1 + 2 * d) * BL:
                                      (tt[d] + 2 * d) * BL]
                            for d in range(2)]
                    # stage-major emission: engine queues alternate f/r so a
                    # stalled instruction never blocks the other chain.
                    pt = []
                    for d in range(2):
                        p = ps3.tile([128, 16], FP, space="PSUM",
                                     tag=f"g{d}", name=f"g{d}_{step}")
                        pt.append(p)
                        for q in range(4):
                            nc.tensor.matmul(
                                p[32 * q:32 * (q + 1), :],
                                lhsT=idh_sb[:, 32 * q:32 * (q + 1)],
                                rhs=xg_sb[d][:, tt[d] * 16:(tt[d] + 1) * 16],
                                start=True, stop=False,
                                tile_position=(0, 32 * q),
                                skip_group_check=True)
                    for d in range(2):
                        for g in range(4):
                            for q in range(4):
                                nc.tensor.matmul(
                                    pt[d][32 * q:32 * (q + 1), g * 4:(g + 1) * 4],
                                    lhsT=whh_sb[d][:, g * 128 + 32 * q:
                                                   g * 128 + 32 * (q + 1)],
                                    rhs=prev[d],
                                    start=False, stop=(g == 3 and q == 3),
                                    tile_position=(0, 32 * q),
                                    skip_group_check=True)
                    th = []
                    for d in range(2):
                        t_ = thpool.tile([128, 16], FP, tag=f"th{d}",
                                         name=f"th{d}_{step}")
                        th.append(t_)
                        nc.scalar.activation(t_[:], pt[d][:], AF.Tanh)
                    # V order a0 b0 c0 | a1 b1 c1 keeps chain d0 from
                    # queuing behind d1's th-dependent ops; tc emitted
                    # right after each dir's c so S packs th0 th1 tc0 tc1.
                    c_new = []
                    tc_t = []
                    for d in range(2):
                        a_t = cellpool.tile([128, BL], FP, tag=f"a{d}",
                                            name=f"a{d}_{step}")
                        b_t = cellpool.tile([128, BL], FP, tag=f"b{d}",
                                            name=f"b{d}_{step}")
                        nc.vector.scalar_tensor_tensor(
                            out=a_t[:], in0=th[d][:, 4:8], scalar=1.0,
                            in1=c_cur[d][:], op0=OP.add, op1=OP.mult)
                        nc.vector.scalar_tensor_tensor(
                            out=b_t[:], in0=th[d][:, 0:4], scalar=1.0,
                            in1=th[d][:, 12:16], op0=OP.add, op1=OP.mult)
                        c_n = cstpool.tile([128, BL], FP, tag=f"c{d}",
                                           name=f"c{d}_{step}")
                        c_new.append(c_n)
                        nc.vector.scalar_tensor_tensor(
                            out=c_n[:], in0=a_t[:], scalar=0.5,
                            in1=b_t[:], op0=OP.mult, op1=OP.add)
                        t_ = cellpool.tile([128, BL], FP, tag=f"tc{d}",
                                           name=f"tc{d}_{step}")
                        tc_t.append(t_)
                        nc.scalar.activation(t_[:], c_n[:], AF.Tanh,
                                             scale=0.5)
                    for d in range(2):
                        nc.vector.scalar_tensor_tensor(
                            out=hs_sb[d][:, tt[d] * BL:(tt[d] + 1) * BL],
                            in0=th[d][:, 8:12], scalar=1.0,
                            in1=tc_t[d][:], op0=OP.add, op1=OP.mult)
                        c_cur[d] = c_new[d]

            # ---- phase 4: emission scores (tags 0..8) -> f128_d ----
            with tc.tile_pool(name="ps4", bufs=3, space="PSUM") as ps4, \
                 tc.tile_pool(name="fsb", bufs=3) as fpool:
                for ch in range(NTILE):
                    pt = ps4.tile([128, JN], FP, space="PSUM")
                    for d in range(2):
                        nc.tensor.matmul(
                            pt[:],
                            lhsT=hs_sb[d][:, ch * 128:(ch + 1) * 128],
                            rhs=wout_sb[d][:],
                            start=(d == 0), stop=(d == 1))
                    fsb = fpool.tile([128, JN], FP)
                    nc.vector.tensor_add(fsb[:], pt[:], bout_sb[:])
                    nc.sync.dma_start(f128_d[ch], fsb[:])

            # ---- phase 5: chunked Viterbi ----
            # partition p = b*NC + cc  (4 seqs x 32 chunks)
            with tc.tile_pool(name="vit", bufs=1) as vp:
                FT = vp.tile([128, CL * JN], FP, name="FT")        # feats [p,(l,j)]
                nc.sync.dma_start(
                    ft2_d.rearrange("b (c th) l j -> c th l b j", th=2),
                    f128_d.rearrange("c (th l b) j -> c th l b j",
                                     th=2, l=CL, b=BL))
                nc.sync.dma_start(
                    FT[:], ft2_d.rearrange("b cc l j -> (b cc) (l j)"))
                a3 = a10_sb[:].rearrange("p (j k) -> p j k", k=KN)  # [p,j,k10]

                # pass 1: chunk matrices P[j,i] stored [p,(j,i)]
                ptA = vp.tile([128, JN * KN], FP, name="ptA")
                ptB = vp.tile([128, JN * KN], FP, name="ptB")
                tmp = vp.tile([128, JN * KN * JN], FP, name="tmp")  # [p,(j,i,k9)]
                # init: P0[j,i] = A[j,i] + f0[j]
                nc.vector.tensor_tensor(
                    out=ptA[:].rearrange("p (j i) -> p j i", i=KN),
                    in0=a3,
                    in1=FT[:, 0:JN].unsqueeze(2).broadcast_to([128, JN, KN]),
                    op=OP.add)
                cur, nxt = ptA, ptB
                for l in range(1, CL):
                    nc.vector.tensor_tensor(
                        out=tmp[:].rearrange("p (j i k) -> p j i k", i=KN, k=JN),
                        in0=a3[:, :, 0:JN].unsqueeze(2)
                            .broadcast_to([128, JN, KN, JN]),
                        in1=cur[:].rearrange("p (k i) -> p k i", i=KN)
                            .transpose([0, 2, 1]).unsqueeze(1)
                            .broadcast_to([128, JN, KN, JN]),
                        op=OP.add)
                    nc.vector.reduce_max(
                        nxt[:].rearrange("p (j i) -> p j i", i=KN),
                        tmp[:].rearrange("p (j i k) -> p j i k", i=KN, k=JN),
                        axis=AX.X)
                    nc.vector.tensor_tensor(
                        out=nxt[:].rearrange("p (j i) -> p j i", i=KN),
                        in0=nxt[:].rearrange("p (j i) -> p j i", i=KN),
                        in1=FT[:, l * JN:(l + 1) * JN].unsqueeze(2)
                            .broadcast_to([128, JN, KN]),
                        op=OP.add)
                    cur, nxt = nxt, cur
                nc.sync.dma_start(p_d, cur[:])

                # pass 2: boundary walk on [BL, *]
                PW = vp.tile([BL, NC * JN * KN], FP, name="PW")
                nc.sync.dma_start(
                    PW[:], p_d.rearrange("(b cc) x -> b (cc x)", b=BL))
                SENT = vp.tile([BL, (NC + 1) * KN], FP, name="SENT")
                nc.vector.memset(SENT[:], NEG)
                nc.vector.memset(SENT[:, START:START + 1], 0.0)
                pwv = PW[:].rearrange("b (cc j i) -> b cc j i", j=JN, i=KN)
                sev = SENT[:].rearrange("b (cc k) -> b cc k", k=KN)
                tw = vp.tile([BL, JN * KN], FP, name="tw")
                for c in range(NC):
                    nc.vector.tensor_tensor(
                        out=tw[:].rearrange("b (o j i) -> b o j i", o=1, i=KN),
                        in0=pwv[:, c:c + 1],
                        in1=sev[:, c:c + 1].unsqueeze(2)
                            .broadcast_to([BL, 1, JN, KN]),
                        op=OP.add)
                    nc.vector.reduce_max(
                        SENT[:, (c + 1) * KN:(c + 1) * KN + JN],
                        tw[:].rearrange("b (o j i) -> b o j i", o=1, i=KN),
                        axis=AX.X)
                # best tag (w-value)
                fin = vp.tile([BL, JN], FP, name="fin")
                nc.vector.tensor_add(fin[:], SENT[:, NC * KN:NC * KN + JN],
                                     ts_sb[:])
                mx8 = vp.tile([BL, 8], FP, name="mx8")
                nc.vector.max(mx8[:], fin[:])
                nc.vector.tensor_scalar(
                    out=fin[:], in0=fin[:], scalar1=mx8[:, 0:1], scalar2=None,
                    op0=OP.is_equal)
                nc.vector.tensor_mul(fin[:], fin[:], wvj_sb[0:BL, :])
                bw = vp.tile([BL, 1], FP, name="bw")
                nc.vector.reduce_max(bw[:], fin[:].rearrange("b (o j) -> b o j", o=1),
                                     axis=AX.X)
                nc.sync.dma_start(
                    s_d, SENT[:, 0:NC * KN].rearrange("b (cc k) -> b cc k",
                                                      k=KN))

                # pass 3: within-chunk scan + pointer extraction (w-values)
                SC = vp.tile([128, KN], FP, name="SC")
                nc.sync.dma_start(
                    SC[:], s_d.rearrange("b cc k -> (b cc) k"))
                PTR = vp.tile([128, CL * JN], FP, name="PTR")
                m3 = vp.tile([128, JN * KN], FP, name="m3")
                mx = vp.tile([128, JN], FP, name="mxs")
                m3v = m3[:].rearrange("p (j k) -> p j k", k=KN)
                for l in range(CL):
                    nc.vector.tensor_tensor(
                        out=m3v,
                        in0=SC[:].unsqueeze(1).broadcast_to([128, JN, KN]),
                        in1=a3, op=OP.add)
                    nc.vector.reduce_max(mx[:], m3v, axis=AX.X)
                    nc.vector.tensor_tensor(
                        out=m3v, in0=m3v,
                        in1=mx[:].unsqueeze(2).broadcast_to([128, JN, KN]),
                        op=OP.is_equal)
                    nc.vector.tensor_tensor(
                        out=m3v, in0=m3v,
                        in1=wvk_sb[:].unsqueeze(1).broadcast_to([128, JN, KN]),
                        op=OP.mult)
                    nc.vector.reduce_max(PTR[:, l * JN:(l + 1) * JN], m3v,
                                         axis=AX.X)
                    nc.vector.tensor_add(SC[:, 0:JN], mx[:],
                                         FT[:, l * JN:(l + 1) * JN])

                # bt pass 1: suffix compositions SUF[l] and chunk map FC
                SUF = vp.tile([128, CL * JN], FP, name="SUF")
                FC = vp.tile([128, JN], FP, name="FC")
                oh = vp.tile([128, JN * JN], FP, name="oh")
                ohv = oh[:].rearrange("p (j i) -> p j i", i=JN)
                nc.vector.tensor_copy(SUF[:, (CL - 1) * JN:CL * JN],
                                      wvj_sb[:])
                for l in range(CL - 2, -2, -1):
                    src = SUF[:, (l + 1) * JN:(l + 2) * JN]
                    dst = FC[:] if l == -1 else SUF[:, l * JN:(l + 1) * JN]
                    nc.vector.tensor_tensor(
                        out=ohv,
                        in0=src.unsqueeze(2).broadcast_to([128, JN, JN]),
                        in1=wvj_sb[:].unsqueeze(1).broadcast_to([128, JN, JN]),
                        op=OP.is_equal)
                    nc.vector.tensor_tensor(
                        out=ohv, in0=ohv,
                        in1=PTR[:, (l + 1) * JN:(l + 2) * JN]
                            .unsqueeze(1).broadcast_to([128, JN, JN]),
                        op=OP.mult)
                    nc.vector.reduce_max(dst, ohv, axis=AX.X)
                nc.sync.dma_start(fc_d, FC[:])

                # tag walk on [BL, *]
                FW = vp.tile([BL, NC * JN], FP, name="FW")
                nc.sync.dma_start(
                    FW[:], fc_d.rearrange("(b cc) j -> b (cc j)", b=BL))
                TE = vp.tile([BL, NC], FP, name="TE")
                ohw = vp.tile([BL, JN], FP, name="ohw")
                scr = vp.tile([BL, JN], FP, name="scr")
                nc.vector.tensor_copy(TE[:, NC - 1:NC], bw[:])
                for c in range(NC - 1, 0, -1):
                    nc.vector.tensor_scalar(
                        out=ohw[:], in0=wvj_sb[0:BL, :],
                        scalar1=TE[:, c:c + 1], scalar2=None, op0=OP.is_equal)
                    nc.vector.scalar_tensor_tensor(
                        out=scr[:], in0=ohw[:], scalar=1.0,
                        in1=FW[:, c * JN:(c + 1) * JN],
                        op0=OP.mult, op1=OP.mult,
                        accum_out=TE[:, c - 1:c])
                nc.sync.dma_start(te_d, TE[:])

                # bt pass 3: apply suffix maps per partition
                TE128 = vp.tile([128, 1], FP, name="TE128")
                nc.sync.dma_start(
                    TE128[:], te_d.rearrange("b (cc x) -> (b cc) x", x=1))
                oh2 = vp.tile([128, JN], FP, name="oh2")
                nc.vector.tensor_scalar(
                    out=oh2[:], in0=wvj_sb[:], scalar1=TE128[:, 0:1],
                    scalar2=None, op0=OP.is_equal)
                big = vp.tile([128, CL * JN], FP, name="bigm")
                nc.vector.tensor_tensor(
                    out=big[:].rearrange("p (l j) -> p l j", j=JN),
                    in0=SUF[:].rearrange("p (l j) -> p l j", j=JN),
                    in1=oh2[:].unsqueeze(1).broadcast_to([128, CL, JN]),
                    op=OP.mult)
                pw16 = vp.tile([128, CL], FP, name="pw16")
                nc.vector.reduce_max(
                    pw16[:], big[:].rearrange("p (l j) -> p l j", j=JN),
                    axis=AX.X)
                pi = vp.tile([128, CL], I32, name="pi")
                nc.vector.tensor_scalar(
                    out=pi[:], in0=pw16[:], scalar1=-1.0, scalar2=float(K - 1),
                    op0=OP.mult, op1=OP.add)
                nc.sync.dma_start(
                    path_out.rearrange("b (cc l) -> (b cc) l", l=CL), pi[:])

    nc.compile()
    return nc


def prep_inputs(sentence, h0, c0, embed, W_ih_f, W_hh_f, b_f, W_ih_r, W_hh_r,
                b_r, W_out, b_out, transitions, T=512):
    """Host-side layout prep. Returns per-core input maps."""
    f32 = np.float32
    perm = np.r_[0:128, 128:256, 384:512, 256:384]  # i,f,g,o -> i,f,o,g
    gs = np.concatenate([np.full(128, s, f32) for s in (0.5, 0.5, 0.5, 1.0)])

    def prep_dir(W_ih, W_hh, b):
        Wi = np.asarray(W_ih, f32)[perm] * gs[:, None]
        bb = np.asarray(b, f32)[perm] * gs
        Wh = np.asarray(W_hh, f32)[perm] * (0.5 * gs)[:, None]
        return Wi.T.copy(), Wh.T.copy(), bb

    wihT_f, whhT_f, be_f = prep_dir(W_ih_f, W_hh_f, b_f)
    wihT_r, whhT_r, be_r = prep_dir(W_ih_r, W_hh_r, b_r)
    w_ihT = np.stack([wihT_f, wihT_r]).astype(np.float16)
    w_hhT = np.stack([whhT_f, whhT_r]).astype(np.float16)
    b_in = np.stack([be_f.reshape(4, 128), be_r.reshape(4, 128)])  # [2,4,128]
    b_in = b_in.reshape(8, 128).T.copy()                           # [128,8]

    Wo = np.asarray(W_out, f32)[0:JN] * 0.5                        # tags 0..8
    w_outT = np.stack([Wo[:, :128].T.copy(),
                       Wo[:, 128:].T.copy()]).astype(np.float16)
    bout_rep = np.tile(np.asarray(b_out, f32)[None, 0:JN], (128, 1))

    tr = np.asarray(transitions, f32)
    a10 = np.tile(tr[0:JN, 0:KN].reshape(1, JN * KN), (128, 1))
    wvk = np.tile((11.0 - np.arange(KN, dtype=f32))[None, :], (128, 1))
    wvj = np.tile((11.0 - np.arange(JN, dtype=f32))[None, :], (128, 1))
    tstop9 = np.tile(tr[STOP, 0:JN][None, :], (BL, 1))
    ident = np.eye(128, dtype=f32)
    embed = np.asarray(embed, f32)
    sentence = np.asarray(sentence)

    maps = []
    for core in range(NCORES):
        sl = sentence[core * BL:(core + 1) * BL, :T].astype(np.int32)
        idx_tm = sl.T.reshape(-1)                       # n = t*BL+b
        idx_in = idx_tm.reshape(-1, 128).T.copy()       # [128, NTILE]
        h_i = 2.0 * np.asarray(h0, f32)[:, core * BL:(core + 1) * BL, :]
        c_i = 2.0 * np.asarray(c0, f32)[:, core * BL:(core + 1) * BL, :]
        maps.append({
            "idx_in": idx_in,
            "embed": embed,
            "w_ihT": w_ihT,
            "w_hhT": w_hhT,
            "b_in": b_in,
            "h_init": np.ascontiguousarray(h_i.transpose(0, 2, 1)).astype(np.float16),
            "c_init": np.ascontiguousarray(c_i.transpose(0, 2, 1)),
            "w_outT": w_outT,
            "bout_rep": bout_rep,
            "ident": ident,
            "identh": ident.astype(np.float16),
            "a10": a10,
            "wvk": wvk,
            "wvj": wvj,
            "tstop9": tstop9,
        })
    return maps


_NC_CACHE = {}


def kernel(sentence, h0, c0, embed, W_ih_f, W_hh_f, b_f, W_ih_r, W_hh_r, b_r,
           W_out, b_out, transitions):
    T = np.asarray(sentence).shape[1]
    if T not in _NC_CACHE:
        _NC_CACHE[T] = build_program(T)
    nc = _NC_CACHE[T]
    maps = prep_inputs(sentence, h0, c0, embed, W_ih_f, W_hh_f, b_f,
                       W_ih_r, W_hh_r, b_r, W_out, b_out, transitions, T=T)
    res = run_bass_kernel_spmd(nc, maps, list(range(NCORES)))
    out = np.concatenate([res.results[i]["path_out"] for i in range(NCORES)], axis=0)
    return out.astype(np.int32)
